# revision 15
# baseline (speedup 1.0000x reference)
"""AVFusion kernel for 8 trn2 NeuronCores — v2.

Per core (data-parallel over bs, 2 batches/core), all activations
transposed (d on partitions as [128, d/128, tokens]).

Math: the 2-way A/V softmax collapses to p = sigmoid((q.kA - q.kV)/sqrt(dk));
x = vV + p*dV per head chunk, so the layer-1 preact is affine in the 8-dim p:
    y1pre[b,s,g] = y0[b,g] + C[b,g] @ p[b,s,g,:]
with y0 = w1@vV + b1 and C[:,h] = w1[:,h-chunk] @ dV[h-chunk].

v2 changes vs v1:
  * Host-side weight folding: wkA=wk@wA, wkVn=-wk@wV, wqS=wq@wS,
    wvA=wv@wA, wvVn=-wv@wV (+ folded bias constants) -> projections become
    single PSUM accumulation chains, 7 device weight matrices instead of 8.
  * AV tokens permuted as tok = b*64 + gl*8 + gc (g = gc*8+gl), which makes
    the C/y0 scatter into the layer-1 stationaries 18+2 strided DMAs
    (one per (h,b)) instead of 144 tiny ones.
  * Weight DMAs split into per-m-chunk pieces issued in consumption order
    on the gpsimd queue; attention path computes per-head as chunks land.
  * p scatter: 64 DMAs (one per (gl,gc), both batches + all heads at once),
    split across the sync and scalar queues.
  * Main loop software-pipelined one block deep (L1(i+1) emitted before
    L2(i)) so relu drains never stall the PE; layer-1 PSUM packs 2 m-chunks
    per bank -> half the relu ACT ops.
"""

import numpy as np

BS, NSEG, NSEN, D, H, DK = 16, 64, 32, 1024, 8, 128
NCORES = 8
BPC = BS // NCORES           # batches per core = 2
TOK_AV = BPC * NSEG          # 128
TOK_S = BPC * NSEN           # 64
TOK_OUT = BPC * NSEN * NSEG  # 4096
KC = D // 128                # 8 d-chunks
GC = 8                       # g's per block
RG = H + 1                   # rows per g in the stationary (8 C + 1 y0)
KB = GC * RG                 # contraction rows per block = 72
BLK = GC * NSEN              # 256 tokens per block
NBLK = TOK_OUT // BLK        # 16 blocks per core
SCALE = 1.0 / np.sqrt(np.float32(DK))

_CACHE = {}

# cst rows
I_CK, I_CD, I_CQ, I_CV, I_B2 = range(5)


def _build_nc():
    import concourse.bass as bass
    import concourse.mybir as mybir
    import concourse.tile as tile
    from concourse import bacc
    from contextlib import ExitStack

    fp32 = mybir.dt.float32
    bf16 = mybir.dt.bfloat16
    AF = mybir.ActivationFunctionType
    ALU = mybir.AluOpType

    nc = bacc.Bacc(None, target_bir_lowering=False)

    # ---- per-core DRAM I/O (host pre-transposed / pre-folded layouts) ----
    AT = nc.dram_tensor("AT", [128, KC, TOK_AV], bf16, kind="ExternalInput")
    VT = nc.dram_tensor("VT", [128, KC, TOK_AV], bf16, kind="ExternalInput")
    ST = nc.dram_tensor("ST", [128, KC, TOK_S], bf16, kind="ExternalInput")
    WNAMES = ["wkA", "wkVn", "wqS", "wvVn", "wvA", "w1f", "w2f"]
    wts = {w: nc.dram_tensor(w, [128, KC, D], bf16, kind="ExternalInput")
           for w in WNAMES}
    CST = nc.dram_tensor("cst", [128, 5, KC], fp32, kind="ExternalInput")
    B1R = nc.dram_tensor("b1r", [1, D], bf16, kind="ExternalInput")
    PCONST = nc.dram_tensor("pconst", [KB, BPC, GC, BLK], bf16,
                            kind="ExternalInput")
    OUT = nc.dram_tensor("OUT", [128, KC, TOK_OUT], bf16,
                         kind="ExternalOutput")

    with tile.TileContext(nc) as tc, ExitStack() as ctx:
        pers = ctx.enter_context(tc.tile_pool(name="pers", bufs=1))
        cst = pers.tile([128, 5, KC], fp32, tag="cst", name="cst")
        b1r = pers.tile([1, D], bf16, tag="b1r", name="b1r")
        ones1 = pers.tile([1, TOK_AV], bf16, tag="ones1", name="ones1")
        pall = pers.tile([KB, BPC, GC, BLK], bf16, tag="pall", name="pall")
        Ccat = pers.tile([KB, NBLK * D], bf16, tag="Ccat", name="Ccat")
        w2sb = pers.tile([128, KC, D], bf16, tag="w2sb", name="w2sb")

        nc.vector.memset(ones1[:], 1.0)

        def csc(row, m):
            # per-partition scalar AP for output chunk m
            return cst[:, row, m:m + 1]

        with tc.tile_pool(name="wpro", bufs=1) as wpro, \
             tc.tile_pool(name="acts", bufs=1) as acts, \
             tc.tile_pool(name="ppsum", bufs=3, space="PSUM") as ppsum, \
             tc.tile_pool(name="cpsum", bufs=3, space="PSUM") as cpsum, \
             tc.tile_pool(name="ypsum", bufs=1, space="PSUM") as ypsum:

            ATs = acts.tile([128, KC, TOK_AV], bf16, tag="ATs", name="ATs")
            VTs = acts.tile([128, KC, TOK_AV], bf16, tag="VTs", name="VTs")
            STs = acts.tile([128, KC, TOK_S], bf16, tag="STs", name="STs")
            nc.sync.dma_start(ATs[:], AT[:])
            nc.sync.dma_start(VTs[:], VT[:])
            nc.sync.dma_start(STs[:], ST[:])
            nc.sync.dma_start(cst[:], CST[:])
            nc.sync.dma_start(b1r[:], B1R[:])
            nc.sync.dma_start(pall[:], PCONST[:])

            wsb = {w: wpro.tile([128, KC, D], bf16, tag=w, name=w)
                   for w in WNAMES[:-1]}
            wsb["w2f"] = w2sb

            # ---- weight chunk DMAs, coarse, in consumption order (gpsimd,
            # SWDGE issue is ~1.15us/DMA so fewer+bigger wins) ----
            def wchunk(w, q, nq):
                cw = D // nq
                sl = slice(q * cw, (q + 1) * cw)
                nc.gpsimd.dma_start(wsb[w][:, :, sl], wts[w][:, :, sl])

            for q in range(4):
                wchunk("wkA", q, 4)
                wchunk("wkVn", q, 4)
                wchunk("wqS", q, 4)
            for q in range(2):
                wchunk("wvVn", q, 2)
                wchunk("wvA", q, 2)
            for q in range(2):
                wchunk("w1f", q, 2)
            for q in range(2):
                wchunk("w2f", q, 2)

            dkT = acts.tile([128, KC, TOK_AV], bf16, tag="dkT", name="dkT")
            qT = acts.tile([128, KC, TOK_S], bf16, tag="qT", name="qT")
            vVT = acts.tile([128, KC, TOK_AV], bf16, tag="vVT", name="vVT")
            dVT = acts.tile([128, KC, TOK_AV], bf16, tag="dVT", name="dVT")
            ctall = acts.tile([128, H, D], bf16, tag="ctall", name="ctall")
            y0tmp = acts.tile([128, D], bf16, tag="y0tmp", name="y0tmp")
            pgs = acts.tile([64, H, BPC, NSEN], bf16, tag="pgs", name="pgs")

            # ---- attention path per head h ----
            for h in range(H):
                sl = slice(h * 128, (h + 1) * 128)
                ps = ppsum.tile([128, TOK_AV], fp32, tag="pp", name="pp")
                for k in range(KC):
                    nc.tensor.matmul(ps[:], wsb["wkA"][:, k, sl],
                                     ATs[:, k, :], start=(k == 0), stop=False)
                for k in range(KC):
                    nc.tensor.matmul(ps[:], wsb["wkVn"][:, k, sl],
                                     VTs[:, k, :], start=False,
                                     stop=(k == KC - 1))
                nc.vector.tensor_scalar_add(dkT[:, h, :], ps[:],
                                            csc(I_CK, h))
                ps2 = ppsum.tile([128, TOK_AV], fp32, tag="pp", name="pp2")
                for k in range(KC):
                    nc.tensor.matmul(ps2[:, :TOK_S], wsb["wqS"][:, k, sl],
                                     STs[:, k, :], start=(k == 0),
                                     stop=(k == KC - 1))
                nc.vector.tensor_scalar_add(qT[:, h, :], ps2[:, :TOK_S],
                                            csc(I_CQ, h))
                for b in range(BPC):
                    lgp = ppsum.tile([64, NSEN], fp32, tag="pp", name="lgp")
                    nc.tensor.matmul(
                        lgp[:],
                        dkT[:, h, b * NSEG:(b + 1) * NSEG],
                        qT[:, h, b * NSEN:(b + 1) * NSEN],
                        start=True, stop=True)
                    nc.scalar.activation(pgs[:, h, b, :], lgp[:], AF.Sigmoid,
                                         scale=float(SCALE))

            # ---- p scatter: one DMA per (gl, gc), sync+scalar queues ----
            for gc in range(GC):
                for gl in range(GC):
                    j = gl * 8 + gc
                    eng = nc.sync if (gc * GC + gl) % 2 == 0 else nc.scalar
                    eng.dma_start(
                        pall[RG * gl:RG * gl + H, :, gc,
                             32 * gl:32 * gl + 32],
                        pgs[j:j + 1, :, :, :])

            # ---- v path per chunk j ----
            for j in range(KC):
                sl = slice(j * 128, (j + 1) * 128)
                psv = ppsum.tile([128, TOK_AV], fp32, tag="pp", name="ppv")
                for k in range(KC):
                    nc.tensor.matmul(psv[:], wsb["wvVn"][:, k, sl],
                                     VTs[:, k, :], start=(k == 0),
                                     stop=(k == KC - 1))
                nc.vector.tensor_scalar(vVT[:, j, :], psv[:], -1.0,
                                        csc(I_CV, j), ALU.mult, ALU.add)
                psd = ppsum.tile([128, TOK_AV], fp32, tag="pp", name="ppd")
                for k in range(KC):
                    nc.tensor.matmul(psd[:], wsb["wvA"][:, k, sl],
                                     ATs[:, k, :], start=(k == 0), stop=False)
                for k in range(KC):
                    nc.tensor.matmul(psd[:], wsb["wvVn"][:, k, sl],
                                     VTs[:, k, :], start=False,
                                     stop=(k == KC - 1))
                nc.vector.tensor_scalar_add(dVT[:, j, :], psd[:],
                                            csc(I_CD, j))

            # ---- C / y0 build, wide-N (one MM per (h, half)) ------------
            psy = ypsum.tile([128, D], fp32, tag="psy", name="psy")
            for hf in range(2):
                sl = slice(hf * 512, (hf + 1) * 512)
                for h in range(H):
                    cps = cpsum.tile([128, 512], fp32, tag="cps", name="cps")
                    nc.tensor.matmul(cps[:], dVT[:, h, :],
                                     wsb["w1f"][:, h, sl],
                                     start=True, stop=True)
                    nc.vector.tensor_copy(ctall[:, h, sl], cps[:])
                for k in range(KC):
                    nc.tensor.matmul(psy[:, sl], vVT[:, k, :],
                                     wsb["w1f"][:, k, sl],
                                     start=(k == 0), stop=False)
                nc.tensor.matmul(psy[:, sl], ones1[:], b1r[:, sl],
                                 start=False, stop=True)
            nc.vector.tensor_copy(y0tmp[:], psy[:])

            # ---- C / y0 scatter into layer-1 stationaries ----------------
            # b=0 half on scalar (lands first, blocks 0-7), b=1 on gpsimd.
            for b in range(BPC):
                eng = nc.scalar if b == 0 else nc.gpsimd
                cs = slice(b * NBLK // BPC * D, (b + 1) * NBLK // BPC * D)
                for h in range(H):
                    eng.dma_start(Ccat[h:KB:RG, cs],
                                  ctall[b * 64:(b + 1) * 64, h, :])
                eng.dma_start(Ccat[H:KB:RG, cs],
                              y0tmp[b * 64:(b + 1) * 64, :])

        # ------- main loop: blocks paired (N=512 layer-2), pipelined -----
        NPAIR = NBLK // 2
        with tc.tile_pool(name="y1p", bufs=2) as y1p, \
             tc.tile_pool(name="obp", bufs=2) as obp, \
             tc.tile_pool(name="f1ps", bufs=4, space="PSUM") as f1ps, \
             tc.tile_pool(name="f2ps", bufs=4, space="PSUM") as f2ps:

            y1s = [None] * NPAIR

            def emit_l1(p):
                # layer 1 for blocks 2p and 2p+1 into one paired y1 tile
                y1 = y1p.tile([128, KC, 2 * BLK], bf16, tag="y1", name="y1")
                y1s[p] = y1
                for half in range(2):
                    i = 2 * p + half
                    b, gc = divmod(i, GC)
                    ts = slice(half * BLK, (half + 1) * BLK)
                    for mp in range(KC // 2):
                        ps = f1ps.tile([128, 2 * BLK], fp32, tag="f1",
                                       name="f1")
                        for mh in range(2):
                            m = 2 * mp + mh
                            nc.tensor.matmul(
                                ps[:, mh * BLK:(mh + 1) * BLK],
                                Ccat[:, i * D + m * 128:i * D + (m + 1) * 128],
                                pall[:, b, gc, :], start=True, stop=True)
                        nc.scalar.activation(
                            y1[:, 2 * mp:2 * mp + 2, ts], ps[:], AF.Relu)

            def emit_l2(p):
                y1 = y1s[p]
                ob = obp.tile([128, KC, 2 * BLK], bf16, tag="ob", name="ob")
                for m in range(KC):
                    sl = slice(m * 128, (m + 1) * 128)
                    ps = f2ps.tile([128, 2 * BLK], fp32, tag="f2", name="f2")
                    for k in range(KC):
                        nc.tensor.matmul(ps[:], w2sb[:, k, sl], y1[:, k, :],
                                         start=(k == 0), stop=(k == KC - 1))
                    nc.vector.tensor_scalar_add(ob[:, m, :], ps[:],
                                                csc(I_B2, m))
                y1s[p] = None
                nc.gpsimd.dma_start(
                    OUT[:, :, 2 * p * BLK:2 * (p + 1) * BLK], ob[:])

            emit_l1(0)
            for p in range(1, NPAIR):
                emit_l1(p)
                emit_l2(p - 1)
            emit_l2(NPAIR - 1)

    nc.finalize()
    return nc


def _prep_core_inputs(inputs, core, folded):
    b0 = core * BPC
    f32 = np.float32

    import ml_dtypes
    bf16 = ml_dtypes.bfloat16

    # AV token permutation: position b*64 + gl*8 + gc holds (b, g=gc*8+gl)
    pos = np.arange(TOK_AV)
    pb = pos // 64
    pr = pos % 64
    pg = (pr % 8) * 8 + pr // 8          # g = gc*8+gl with gl=pr//8, gc=pr%8

    def t_act_av(x):
        xp = x[b0 + pb, pg]              # (128, D) in pi order
        flat = xp.reshape(TOK_AV, KC, 128)
        return np.ascontiguousarray(flat.transpose(2, 1, 0)).astype(bf16)

    def t_act_s(x):
        flat = np.ascontiguousarray(x[b0:b0 + BPC]).reshape(TOK_S, KC, 128)
        return np.ascontiguousarray(flat.transpose(2, 1, 0)).astype(bf16)

    m = {
        "AT": t_act_av(inputs["A"]),
        "VT": t_act_av(inputs["V"]),
        "ST": t_act_s(inputs["S"]),
    }
    m.update(folded)
    return m


def _prep_folded(inputs):
    """Core-independent folded weights/constants (computed once)."""
    f32 = np.float32
    import ml_dtypes
    bf16 = ml_dtypes.bfloat16

    def t_w(w):
        wt = np.ascontiguousarray(np.asarray(w, f32).T).reshape(KC, 128, D)
        return np.ascontiguousarray(wt.transpose(1, 0, 2)).astype(bf16)

    def t_b(b):
        return np.ascontiguousarray(np.asarray(b, f32).reshape(KC, 128).T,
                                    dtype=f32)

    wA, wV, wS = inputs["wA"], inputs["wV"], inputs["wS"]
    wq, wk, wv = inputs["wq"], inputs["wk"], inputs["wv"]
    w1, w2 = inputs["w1"], inputs["w2"]
    bA, bV, bS = inputs["bA"], inputs["bV"], inputs["bS"]
    bq, bv = inputs["bq"], inputs["bv"]
    b1, b2 = inputs["b1"], inputs["b2"]

    m = {
        "wkA": t_w(wk @ wA),
        "wkVn": t_w(-(wk @ wV)),
        "wqS": t_w(wq @ wS),
        "wvVn": t_w(-(wv @ wV)),
        "wvA": t_w(wv @ wA),
        "w1f": t_w(w1),
        "w2f": t_w(w2),
    }
    cst = np.stack([
        t_b(wk @ (bA - bV)),
        t_b(wv @ (bA - bV)),
        t_b(wq @ bS + bq),
        t_b(wv @ bV + bv),
        t_b(b2),
    ], axis=1)                           # [128, 5, KC]
    m["cst"] = np.ascontiguousarray(cst)
    m["b1r"] = np.ascontiguousarray(b1.reshape(1, D)).astype(bf16)
    pc = np.zeros((KB, BPC, GC, BLK), dtype=bf16)
    for gl in range(GC):
        pc[RG * gl + H, :, :, 32 * gl:32 * gl + 32] = 1.0
    m["pconst"] = pc
    return m


def kernel(**inputs):
    import os
    from concourse.bass_utils import run_bass_kernel_spmd

    inputs = {k: np.asarray(v, dtype=np.float32) for k, v in inputs.items()}
    if "nc" not in _CACHE:
        _CACHE["nc"] = _build_nc()
    nc = _CACHE["nc"]

    folded = _prep_folded(inputs)
    in_maps = [_prep_core_inputs(inputs, c, folded) for c in range(NCORES)]
    trace = os.environ.get("TRACE", "0") == "1"
    res = run_bass_kernel_spmd(nc, in_maps, core_ids=list(range(NCORES)),
                               trace=trace)
    _CACHE["last_results"] = res

    out = np.empty((BS, NSEN, NSEG, D), dtype=np.float32)
    for c in range(NCORES):
        oc = res.results[c]["OUT"].astype(np.float32)
        # tok = (b, gc, gl, s); g = gc*8 + gl
        oc = oc.reshape(128, KC, BPC, GC, GC, NSEN)
        oc = oc.transpose(2, 5, 3, 4, 1, 0)    # (b, s, gc, gl, k, dd)
        out[c * BPC:(c + 1) * BPC] = oc.reshape(BPC, NSEN, NSEG, D)
    return out


# revision 24
# speedup vs baseline: 1.1914x; 1.1914x over previous
"""AVFusion kernel for 8 trn2 NeuronCores — v2.

Per core (data-parallel over bs, 2 batches/core), all activations
transposed (d on partitions as [128, d/128, tokens]).

Math: the 2-way A/V softmax collapses to p = sigmoid((q.kA - q.kV)/sqrt(dk));
x = vV + p*dV per head chunk, so the layer-1 preact is affine in the 8-dim p:
    y1pre[b,s,g] = y0[b,g] + C[b,g] @ p[b,s,g,:]
with y0 = w1@vV + b1 and C[:,h] = w1[:,h-chunk] @ dV[h-chunk].

v2 changes vs v1:
  * Host-side weight folding: wkA=wk@wA, wkVn=-wk@wV, wqS=wq@wS,
    wvA=wv@wA, wvVn=-wv@wV (+ folded bias constants) -> projections become
    single PSUM accumulation chains, 7 device weight matrices instead of 8.
  * AV tokens permuted as tok = b*64 + gl*8 + gc (g = gc*8+gl), which makes
    the C/y0 scatter into the layer-1 stationaries 18+2 strided DMAs
    (one per (h,b)) instead of 144 tiny ones.
  * Weight DMAs split into per-m-chunk pieces issued in consumption order
    on the gpsimd queue; attention path computes per-head as chunks land.
  * p scatter: 64 DMAs (one per (gl,gc), both batches + all heads at once),
    split across the sync and scalar queues.
  * Main loop software-pipelined one block deep (L1(i+1) emitted before
    L2(i)) so relu drains never stall the PE; layer-1 PSUM packs 2 m-chunks
    per bank -> half the relu ACT ops.
"""

import numpy as np

BS, NSEG, NSEN, D, H, DK = 16, 64, 32, 1024, 8, 128
NCORES = 8
BPC = BS // NCORES           # batches per core = 2
TOK_AV = BPC * NSEG          # 128
TOK_S = BPC * NSEN           # 64
TOK_OUT = BPC * NSEN * NSEG  # 4096
KC = D // 128                # 8 d-chunks
GC = 8                       # g's per block
RG = H + 1                   # rows per g in the stationary (8 C + 1 y0)
KB = GC * RG                 # contraction rows per block = 72
BLK = GC * NSEN              # 256 tokens per block
NBLK = TOK_OUT // BLK        # 16 blocks per core
SCALE = 1.0 / np.sqrt(np.float32(DK))

_CACHE = {}

# cst rows
I_CK, I_CD, I_CQ, I_CV, I_B2 = range(5)


def _build_nc():
    import concourse.bass as bass
    import concourse.mybir as mybir
    import concourse.tile as tile
    from concourse import bacc
    from contextlib import ExitStack

    fp32 = mybir.dt.float32
    bf16 = mybir.dt.bfloat16
    AF = mybir.ActivationFunctionType
    ALU = mybir.AluOpType

    nc = bacc.Bacc(None, target_bir_lowering=False)

    # ---- per-core DRAM I/O (host pre-transposed / pre-folded layouts) ----
    AT = nc.dram_tensor("AT", [128, KC, TOK_AV], bf16, kind="ExternalInput")
    VT = nc.dram_tensor("VT", [128, KC, TOK_AV], bf16, kind="ExternalInput")
    ST = nc.dram_tensor("ST", [128, KC, TOK_S], bf16, kind="ExternalInput")
    WNAMES = ["wkA", "wkVn", "wqS", "wvVn", "wvA", "w1f", "w2f"]
    wts = {w: nc.dram_tensor(w, [128, KC, D], bf16, kind="ExternalInput")
           for w in WNAMES}
    CST = nc.dram_tensor("cst", [128, 5, KC], fp32, kind="ExternalInput")
    B1R = nc.dram_tensor("b1r", [1, D], bf16, kind="ExternalInput")
    ONESD = nc.dram_tensor("onesd", [1, BPC * GC * 32], bf16,
                           kind="ExternalInput")
    OUT = nc.dram_tensor("OUT", [128, KC, TOK_OUT], bf16,
                         kind="ExternalOutput")

    with tile.TileContext(nc) as tc, ExitStack() as ctx:
        pers = ctx.enter_context(tc.tile_pool(name="pers", bufs=1))
        cst = pers.tile([128, 5, KC], fp32, tag="cst", name="cst")
        b1r = pers.tile([1, D], bf16, tag="b1r", name="b1r")
        ones1 = pers.tile([1, TOK_AV], bf16, tag="ones1", name="ones1")
        pall = pers.tile([KB, BPC, GC, BLK], bf16, tag="pall", name="pall")
        Ccat = pers.tile([KB, NBLK * D], bf16, tag="Ccat", name="Ccat")
        w2sb = pers.tile([128, KC, D], bf16, tag="w2sb", name="w2sb")

        nc.vector.memset(ones1[:], 1.0)
        # pall: memset zeros; block-diagonal ones rows via tiny DMAs
        # (engines cannot address partition offsets like 9*gl+8, DMA can)
        nc.vector.memset(pall[:], 0.0)
        for gl in range(GC):
            nc.sync.dma_start(
                pall[RG * gl + H:RG * gl + H + 1, :, :,
                     32 * gl:32 * gl + 32],
                ONESD[:])

        def csc(row, m):
            # per-partition scalar AP for output chunk m
            return cst[:, row, m:m + 1]

        with tc.tile_pool(name="wpro", bufs=1) as wpro, \
             tc.tile_pool(name="acts", bufs=1) as acts, \
             tc.tile_pool(name="ppsum", bufs=3, space="PSUM") as ppsum, \
             tc.tile_pool(name="cpsum", bufs=3, space="PSUM") as cpsum, \
             tc.tile_pool(name="ypsum", bufs=1, space="PSUM") as ypsum:

            ATs = acts.tile([128, KC, TOK_AV], bf16, tag="ATs", name="ATs")
            VTs = acts.tile([128, KC, TOK_AV], bf16, tag="VTs", name="VTs")
            STs = acts.tile([128, KC, TOK_S], bf16, tag="STs", name="STs")
            nc.sync.dma_start(ATs[:, :4, :], AT[:, :4, :])
            nc.sync.dma_start(VTs[:, :4, :], VT[:, :4, :])
            nc.sync.dma_start(ATs[:, 4:, :], AT[:, 4:, :])
            nc.sync.dma_start(VTs[:, 4:, :], VT[:, 4:, :])
            nc.sync.dma_start(STs[:], ST[:])
            nc.sync.dma_start(cst[:], CST[:])
            nc.sync.dma_start(b1r[:], B1R[:])

            wsb = {w: wpro.tile([128, KC, D], bf16, tag=w, name=w)
                   for w in WNAMES[:-1]}
            wsb["w2f"] = w2sb

            # ---- weight chunk DMAs, coarse, in consumption order (gpsimd,
            # SWDGE issue is ~1.15us/DMA so fewer+bigger wins) ----
            def wchunk(w, q, nq):
                cw = D // nq
                sl = slice(q * cw, (q + 1) * cw)
                nc.gpsimd.dma_start(wsb[w][:, :, sl], wts[w][:, :, sl])

            for q in range(4):
                wchunk("wkA", q, 4)
                wchunk("wkVn", q, 4)
                wchunk("wqS", q, 4)
            for q in range(2):
                wchunk("wvVn", q, 2)
                wchunk("wvA", q, 2)
            for q in range(2):
                wchunk("w1f", q, 2)
            for q in range(2):
                wchunk("w2f", q, 2)

            dkT = acts.tile([128, KC, TOK_AV], bf16, tag="dkT", name="dkT")
            qT = acts.tile([128, KC, TOK_S], bf16, tag="qT", name="qT")
            vVT = acts.tile([128, KC, TOK_AV], bf16, tag="vVT", name="vVT")
            dVT = acts.tile([128, KC, TOK_AV], bf16, tag="dVT", name="dVT")
            ctall = acts.tile([128, H, D], bf16, tag="ctall", name="ctall")
            y0tmp = acts.tile([128, D], bf16, tag="y0tmp", name="y0tmp")
            pgs = acts.tile([64, H, BPC, NSEN], bf16, tag="pgs", name="pgs")

            # ---- attention path per head h ----
            for h in range(H):
                sl = slice(h * 128, (h + 1) * 128)
                ps = ppsum.tile([128, TOK_AV], fp32, tag="pp", name="pp")
                for k in range(KC):
                    nc.tensor.matmul(ps[:], wsb["wkA"][:, k, sl],
                                     ATs[:, k, :], start=(k == 0), stop=False)
                for k in range(KC):
                    nc.tensor.matmul(ps[:], wsb["wkVn"][:, k, sl],
                                     VTs[:, k, :], start=False,
                                     stop=(k == KC - 1))
                nc.vector.tensor_scalar_add(dkT[:, h, :], ps[:],
                                            csc(I_CK, h))
                ps2 = ppsum.tile([128, TOK_AV], fp32, tag="pp", name="pp2")
                for k in range(KC):
                    nc.tensor.matmul(ps2[:, :TOK_S], wsb["wqS"][:, k, sl],
                                     STs[:, k, :], start=(k == 0),
                                     stop=(k == KC - 1))
                nc.vector.tensor_scalar_add(qT[:, h, :], ps2[:, :TOK_S],
                                            csc(I_CQ, h))
                for b in range(BPC):
                    lgp = ppsum.tile([64, NSEN], fp32, tag="pp", name="lgp")
                    nc.tensor.matmul(
                        lgp[:],
                        dkT[:, h, b * NSEG:(b + 1) * NSEG],
                        qT[:, h, b * NSEN:(b + 1) * NSEN],
                        start=True, stop=True)
                    nc.scalar.activation(pgs[:, h, b, :], lgp[:], AF.Sigmoid,
                                         scale=float(SCALE))

            # ---- p scatter: one DMA per (gl, gc), all three queues ----
            for gc in range(GC):
                for gl in range(GC):
                    j = gl * 8 + gc
                    eng = [nc.sync, nc.scalar, nc.gpsimd][(gc * GC + gl) % 3]
                    eng.dma_start(
                        pall[RG * gl:RG * gl + H, :, gc,
                             32 * gl:32 * gl + 32],
                        pgs[j:j + 1, :, :, :])

            # ---- v path per chunk j ----
            for j in range(KC):
                sl = slice(j * 128, (j + 1) * 128)
                psv = ppsum.tile([128, TOK_AV], fp32, tag="pp", name="ppv")
                for k in range(KC):
                    nc.tensor.matmul(psv[:], wsb["wvVn"][:, k, sl],
                                     VTs[:, k, :], start=(k == 0),
                                     stop=(k == KC - 1))
                nc.vector.tensor_scalar(vVT[:, j, :], psv[:], -1.0,
                                        csc(I_CV, j), ALU.mult, ALU.add)
                psd = ppsum.tile([128, TOK_AV], fp32, tag="pp", name="ppd")
                for k in range(KC):
                    nc.tensor.matmul(psd[:], wsb["wvA"][:, k, sl],
                                     ATs[:, k, :], start=(k == 0), stop=False)
                for k in range(KC):
                    nc.tensor.matmul(psd[:], wsb["wvVn"][:, k, sl],
                                     VTs[:, k, :], start=False,
                                     stop=(k == KC - 1))
                nc.vector.tensor_scalar_add(dVT[:, j, :], psd[:],
                                            csc(I_CD, j))

            # ---- C / y0 build, wide-N (one MM per (h, half)) ------------
            psy = ypsum.tile([128, D], fp32, tag="psy", name="psy")
            for hf in range(2):
                sl = slice(hf * 512, (hf + 1) * 512)
                for h in range(H):
                    cps = cpsum.tile([128, 512], fp32, tag="cps", name="cps")
                    nc.tensor.matmul(cps[:], dVT[:, h, :],
                                     wsb["w1f"][:, h, sl],
                                     start=True, stop=True)
                    nc.vector.tensor_copy(ctall[:, h, sl], cps[:])
                for k in range(KC):
                    nc.tensor.matmul(psy[:, sl], vVT[:, k, :],
                                     wsb["w1f"][:, k, sl],
                                     start=(k == 0), stop=False)
                nc.tensor.matmul(psy[:, sl], ones1[:], b1r[:, sl],
                                 start=False, stop=True)
            nc.vector.tensor_copy(y0tmp[:], psy[:])

            # ---- C / y0 scatter into layer-1 stationaries ----------------
            # b=0 half split sync/scalar (lands first, unblocks blocks 0-7);
            # b=1 half on gpsimd.
            for b in range(BPC):
                cs = slice(b * NBLK // BPC * D, (b + 1) * NBLK // BPC * D)
                for h in range(H):
                    eng = (nc.gpsimd if b else
                           (nc.sync if h % 2 == 0 else nc.scalar))
                    eng.dma_start(Ccat[h:KB:RG, cs],
                                  ctall[b * 64:(b + 1) * 64, h, :])
                eng = nc.gpsimd if b else nc.scalar
                eng.dma_start(Ccat[H:KB:RG, cs],
                              y0tmp[b * 64:(b + 1) * 64, :])

        # ------- main loop: blocks paired (N=512 layer-2), pipelined -----
        NPAIR = NBLK // 2
        with tc.tile_pool(name="y1p", bufs=2) as y1p, \
             tc.tile_pool(name="obp", bufs=2) as obp, \
             tc.tile_pool(name="f1ps", bufs=4, space="PSUM") as f1ps, \
             tc.tile_pool(name="f2ps", bufs=4, space="PSUM") as f2ps:

            y1s = [None] * NPAIR

            def emit_l1(p):
                # layer 1 for blocks 2p and 2p+1 into one paired y1 tile
                y1 = y1p.tile([128, KC, 2 * BLK], bf16, tag="y1", name="y1")
                y1s[p] = y1
                for half in range(2):
                    i = 2 * p + half
                    b, gc = divmod(i, GC)
                    ts = slice(half * BLK, (half + 1) * BLK)
                    for mp in range(KC // 2):
                        ps = f1ps.tile([128, 2 * BLK], fp32, tag="f1",
                                       name="f1")
                        for mh in range(2):
                            m = 2 * mp + mh
                            nc.tensor.matmul(
                                ps[:, mh * BLK:(mh + 1) * BLK],
                                Ccat[:, i * D + m * 128:i * D + (m + 1) * 128],
                                pall[:, b, gc, :], start=True, stop=True)
                        nc.scalar.activation(
                            y1[:, 2 * mp:2 * mp + 2, ts], ps[:], AF.Relu)

            def emit_l2(p):
                y1 = y1s[p]
                ob = obp.tile([128, KC, 2 * BLK], bf16, tag="ob", name="ob")
                for m in range(KC):
                    sl = slice(m * 128, (m + 1) * 128)
                    ps = f2ps.tile([128, 2 * BLK], fp32, tag="f2", name="f2")
                    for k in range(KC):
                        nc.tensor.matmul(ps[:], w2sb[:, k, sl], y1[:, k, :],
                                         start=(k == 0), stop=(k == KC - 1))
                    nc.vector.tensor_scalar_add(ob[:, m, :], ps[:],
                                                csc(I_B2, m))
                y1s[p] = None
                nc.gpsimd.dma_start(
                    OUT[:, :, 2 * p * BLK:2 * (p + 1) * BLK], ob[:])

            emit_l1(0)
            for p in range(1, NPAIR):
                emit_l1(p)
                emit_l2(p - 1)
            emit_l2(NPAIR - 1)

    nc.finalize()
    return nc


def _prep_core_inputs(inputs, core, folded):
    b0 = core * BPC
    f32 = np.float32

    import ml_dtypes
    bf16 = ml_dtypes.bfloat16

    # AV token permutation: position b*64 + gl*8 + gc holds (b, g=gc*8+gl)
    pos = np.arange(TOK_AV)
    pb = pos // 64
    pr = pos % 64
    pg = (pr % 8) * 8 + pr // 8          # g = gc*8+gl with gl=pr//8, gc=pr%8

    def t_act_av(x):
        xp = x[b0 + pb, pg]              # (128, D) in pi order
        flat = xp.reshape(TOK_AV, KC, 128)
        return np.ascontiguousarray(flat.transpose(2, 1, 0)).astype(bf16)

    def t_act_s(x):
        flat = np.ascontiguousarray(x[b0:b0 + BPC]).reshape(TOK_S, KC, 128)
        return np.ascontiguousarray(flat.transpose(2, 1, 0)).astype(bf16)

    m = {
        "AT": t_act_av(inputs["A"]),
        "VT": t_act_av(inputs["V"]),
        "ST": t_act_s(inputs["S"]),
    }
    m.update(folded)
    return m


def _prep_folded(inputs):
    """Core-independent folded weights/constants (computed once)."""
    f32 = np.float32
    import ml_dtypes
    bf16 = ml_dtypes.bfloat16

    def t_w(w):
        wt = np.ascontiguousarray(np.asarray(w, f32).T).reshape(KC, 128, D)
        return np.ascontiguousarray(wt.transpose(1, 0, 2)).astype(bf16)

    def t_b(b):
        return np.ascontiguousarray(np.asarray(b, f32).reshape(KC, 128).T,
                                    dtype=f32)

    wA, wV, wS = inputs["wA"], inputs["wV"], inputs["wS"]
    wq, wk, wv = inputs["wq"], inputs["wk"], inputs["wv"]
    w1, w2 = inputs["w1"], inputs["w2"]
    bA, bV, bS = inputs["bA"], inputs["bV"], inputs["bS"]
    bq, bv = inputs["bq"], inputs["bv"]
    b1, b2 = inputs["b1"], inputs["b2"]

    m = {
        "wkA": t_w(wk @ wA),
        "wkVn": t_w(-(wk @ wV)),
        "wqS": t_w(wq @ wS),
        "wvVn": t_w(-(wv @ wV)),
        "wvA": t_w(wv @ wA),
        "w1f": t_w(w1),
        "w2f": t_w(w2),
    }
    cst = np.stack([
        t_b(wk @ (bA - bV)),
        t_b(wv @ (bA - bV)),
        t_b(wq @ bS + bq),
        t_b(wv @ bV + bv),
        t_b(b2),
    ], axis=1)                           # [128, 5, KC]
    m["cst"] = np.ascontiguousarray(cst)
    m["b1r"] = np.ascontiguousarray(b1.reshape(1, D)).astype(bf16)
    m["onesd"] = np.ones((1, BPC * GC * 32), dtype=bf16)
    return m


def kernel(**inputs):
    import os
    from concourse.bass_utils import run_bass_kernel_spmd

    inputs = {k: np.asarray(v, dtype=np.float32) for k, v in inputs.items()}
    if "nc" not in _CACHE:
        _CACHE["nc"] = _build_nc()
    nc = _CACHE["nc"]

    folded = _prep_folded(inputs)
    in_maps = [_prep_core_inputs(inputs, c, folded) for c in range(NCORES)]
    trace = os.environ.get("TRACE", "0") == "1"
    res = run_bass_kernel_spmd(nc, in_maps, core_ids=list(range(NCORES)),
                               trace=trace)
    _CACHE["last_results"] = res

    out = np.empty((BS, NSEN, NSEG, D), dtype=np.float32)
    for c in range(NCORES):
        oc = res.results[c]["OUT"].astype(np.float32)
        # tok = (b, gc, gl, s); g = gc*8 + gl
        oc = oc.reshape(128, KC, BPC, GC, GC, NSEN)
        oc = oc.transpose(2, 5, 3, 4, 1, 0)    # (b, s, gc, gl, k, dd)
        out[c * BPC:(c + 1) * BPC] = oc.reshape(BPC, NSEN, NSEG, D)
    return out
